# revision 2
# baseline (speedup 1.0000x reference)
import sys

sys.path.insert(0, "/opt/trn_rl_repo")

import math

import numpy as np

import concourse.bacc as bacc
import concourse.bass as bass
import concourse.mybir as mybir
import concourse.tile as tile
from concourse.bass import ds, ts
from concourse.bass_utils import run_bass_kernel_spmd
from concourse.masks import make_identity

B, C, D = 4096, 10000, 64
NCORES = 8
CS = C // NCORES            # 1250 classes per core
NBLK = B // 128             # 32 row blocks of 128
PA = 125                    # partition rows per all_embs block
JA = CS // PA               # 10
MARGIN = 0.1
EM = math.exp(MARGIN)
EPS_T = 1.0 - 1e-5          # sqrt arg = tau0^2 - EPS_T*W^2; z~1 noise gives
                            # sqrt(<0)=NaN which the DVE max drops to u=1
NGRP = 8                    # phase-B ln instructions (each spans all blocks)
LNW = 160 // NGRP           # inner cols per ln instruction
P1W = 640                   # 512 + 113 merged pairs + 15 pad cols of 1.0
SUBP = ((0, 512), (512, 512), (1024, 226))   # psum sub-panels (1 bank each)

F32 = mybir.dt.float32
F32R = mybir.dt.float32r
BF16 = mybir.dt.bfloat16
AF = mybir.ActivationFunctionType
ALU = mybir.AluOpType
PSUM = bass.MemorySpace.PSUM

_CACHE = {}

# ---------------------------------------------------------------------------
# custom DVE ops
# ---------------------------------------------------------------------------


def _register_custom_ops():
    import concourse.dve_ops as dve_ops
    from concourse.dve_ops import OPS, CUSTOM_DVE_SPECS, DveOp
    from concourse.dve_spec import Spec, Src0, Src1, C0, One, maxx, sq, lower
    from concourse.dve_uop import DveOpSpec
    from concourse.dve_table_gen import dve_ver_for

    if "SUBCLAMP_W_PEH" in CUSTOM_DVE_SPECS:
        return (
            dve_ops._PEH_SQUARE_BIAS,  # type: ignore[attr-defined]
            dve_ops._PEH_CLAMP_MERGE,  # type: ignore[attr-defined]
        )

    def mk(name, spec, rd1):
        row = dve_ops._CUSTOM_DVE_ROW_BASE + len(OPS)
        shas = {}
        for ver in ("v3", "v4"):
            try:
                tmp = DveOpSpec(
                    name=name, opcode=row, uops=lower(spec, ver=ver), rd1_en=rd1
                )
                shas[ver] = tmp.sha(ver)
            except Exception:
                pass
        op = DveOp(name, spec, subdim=False, uops_sha=shas)
        OPS.append(op)
        CUSTOM_DVE_SPECS[name] = spec
        dve_ops._SUB_OPCODE_FOR_NAME[name] = row
        return op

    sq_bias = mk(
        "SQUARE_BIAS_PEH",
        Spec(
            body=sq(Src0) - C0,
            reference=lambda in0, in1, s0, s1, imm2: (
                in0.astype(np.float32) * in0.astype(np.float32) - s0
            ),
        ),
        rd1=False,
    )

    def _sc_ref(in0, in1, s0, s1, imm2):
        v = (in0.astype(np.float32) - in1.astype(np.float32)) * s0
        return np.maximum(np.nan_to_num(v, nan=0.0), 1.0)

    clamp_merge = mk(
        "SUBCLAMP_W_PEH",
        Spec(body=maxx((Src0 - Src1) * C0, One), reference=_sc_ref),
        rd1=True,
    )
    dve_ops._PEH_SQUARE_BIAS = sq_bias  # type: ignore[attr-defined]
    dve_ops._PEH_CLAMP_MERGE = clamp_merge  # type: ignore[attr-defined]
    return sq_bias, clamp_merge


def _patch_act_tables():
    """Pin Square/Sqrt to sqrt_and_others and Ln to natural_log by removing
    them from every other set. Only membership changes; list order (and so
    act_func_set_id) is untouched, and the pinned sets genuinely contain the
    functions on hardware."""
    if getattr(bacc, "_peh_act_patch", False):
        return
    orig = bacc.get_activation_tables

    def patched(arch):
        tabs = {k: set(v) for k, v in orig(arch).items()}
        for name, funcs in tabs.items():
            if name != "sqrt_and_others":
                funcs.discard(AF.Square)
                funcs.discard(AF.Sqrt)
            if name != "natural_log":
                funcs.discard(AF.Ln)
        return tabs

    bacc.get_activation_tables = patched
    bacc._peh_act_patch = True


# ---------------------------------------------------------------------------
# kernel body
# ---------------------------------------------------------------------------


def _build():
    sq_bias, clamp_merge = _register_custom_ops()
    _patch_act_tables()
    nc = bacc.Bacc(None, target_bir_lowering=False)
    pred_d = nc.declare_dram_parameter("pred", [B, D], F32, isOutput=False)
    targ_d = nc.declare_dram_parameter("targ", [B, D], F32, isOutput=False)
    alls_d = nc.declare_dram_parameter("alls", [CS, D], F32, isOutput=False)
    out_d = nc.declare_dram_parameter("partial", [128, 1], F32, isOutput=True)

    with tile.TileContext(nc) as tc:
        _body(nc, tc, pred_d, targ_d, alls_d, out_d, sq_bias, clamp_merge)
    nc.compile()
    return nc


def _body(nc, tc, pred_d, targ_d, alls_d, out_d, sq_bias, clamp_merge):
    with (
        tc.tile_pool(name="persist", bufs=1) as persist,
        tc.tile_pool(name="prep", bufs=1) as prep,
    ):
        # The 67-row augmented matmul gives z = cosh(d) per (row, class).
        # sbar = sqrt(z^2 - EPS_T) ~ sinh(d), and one fused DVE op computes
        # u = max(W*(z - sbar), 1) = max(e^{g-d}, 1), so each hinge term is
        # ln(u); products of u's shrink the final Ln pass 8x.
        phatT = persist.tile([128, B], BF16)
        ahatT = persist.tile([67, CS], BF16)
        hacc = persist.tile([128, NGRP], F32)
        ident = persist.tile([128, 128], BF16)
        p3buf = persist.tile([128, NBLK, 160], BF16)

        make_identity(nc, ident[:])
        biasm1 = persist.tile([128, 1], F32)
        nc.vector.memset(biasm1[:], -(1.0 - 2e-7))
        biasme = persist.tile([128, 1], F32)
        nc.vector.memset(biasme[:], -EPS_T)

        # ---------------- loads ----------------
        prednat = prep.tile([128, NBLK, D], F32)
        targnat = prep.tile([128, NBLK, D], F32)
        allnat = prep.tile([PA, JA, D], F32)
        nc.sync.dma_start(allnat[:], alls_d[:].rearrange("(j p) d -> p j d", p=PA))
        nc.sync.dma_start(prednat[:], pred_d[:].rearrange("(j p) d -> p j d", p=128))
        nc.scalar.dma_start(targnat[:], targ_d[:].rearrange("(j p) d -> p j d", p=128))

        with tc.tile_pool(name="prep_ps", bufs=2, space=PSUM) as prep_ps:
            # ---------------- ahat path (gates all matmuls) ----------------
            asq = prep.tile([PA, JA, D], F32)
            an = prep.tile([PA, JA], F32)
            beta = prep.tile([PA, JA], F32)
            tmpa = prep.tile([PA, JA], F32)
            nc.vector.tensor_mul(asq[:], allnat[:], allnat[:])
            nc.vector.tensor_reduce(an[:], asq[:], mybir.AxisListType.X, ALU.add)
            nc.vector.tensor_scalar(tmpa[:], an[:], -1.0, 1.0, ALU.mult, ALU.add)
            nc.vector.reciprocal(beta[:], tmpa[:])

            ahatnat = prep.tile([PA, JA, 67], BF16)
            for j in range(JA):
                nc.vector.tensor_scalar_mul(
                    ahatnat[:, j, 0:64], allnat[:, j, :], beta[:, ds(j, 1)]
                )
            nc.vector.tensor_copy(ahatnat[:, :, 64], beta[:])
            nc.vector.tensor_mul(tmpa[:], beta[:], an[:])
            nc.vector.tensor_copy(ahatnat[:, :, 65], tmpa[:])
            nc.vector.memset(ahatnat[:, :, 66], 1.0)
            for jj in range(JA):
                pt = prep_ps.tile([67, 128], BF16)
                nc.tensor.transpose(
                    pt[0:67, 0:PA], ahatnat[:, jj, :], ident[0:PA, 0:PA]
                )
                nc.vector.tensor_copy(ahatT[:, ts(jj, PA)], pt[0:67, 0:PA])

            # ---------------- pn/tn, alpha, W = e^{dc+m} ----------------
            sq = prep.tile([128, NBLK, D], F32)
            pn = prep.tile([128, NBLK], F32)
            tn = prep.tile([128, NBLK], F32)
            alpha = prep.tile([128, NBLK], F32)
            alphat = prep.tile([128, NBLK], F32)
            tmp = prep.tile([128, NBLK], F32)

            nc.vector.tensor_mul(sq[:], prednat[:], prednat[:])
            nc.vector.tensor_reduce(pn[:], sq[:], mybir.AxisListType.X, ALU.add)
            nc.vector.tensor_scalar(tmp[:], pn[:], -1.0, 1.0, ALU.mult, ALU.add)
            nc.vector.reciprocal(alpha[:], tmp[:])

            sqt = prep.tile([128, NBLK, D], F32, name="sqt")
            nc.gpsimd.tensor_mul(sqt[:], targnat[:], targnat[:])
            nc.vector.tensor_reduce(tn[:], sqt[:], mybir.AxisListType.X, ALU.add)
            nc.vector.tensor_scalar(tmp[:], tn[:], -1.0, 1.0, ALU.mult, ALU.add)
            nc.vector.reciprocal(alphat[:], tmp[:])

            s2c = prep.tile([128, NBLK], F32)
            zc = prep.tile([128, NBLK], F32)
            zzc = prep.tile([128, NBLK], F32)
            rc = prep.tile([128, NBLK], F32)
            wv = prep.tile([128, NBLK], F32)
            nc.vector.tensor_sub(sq[:], prednat[:], targnat[:])
            nc.vector.tensor_mul(sq[:], sq[:], sq[:])
            nc.vector.tensor_reduce(s2c[:], sq[:], mybir.AxisListType.X, ALU.add)
            nc.vector.tensor_mul(s2c[:], s2c[:], alpha[:])
            nc.vector.tensor_mul(s2c[:], s2c[:], alphat[:])
            nc.vector.tensor_scalar(zc[:], s2c[:], 2.0, 1.0, ALU.mult, ALU.add)
            nc.vector.tensor_mul(zzc[:], zc[:], zc[:])
            nc.scalar.activation(rc[:], zzc[:], AF.Sqrt, bias=biasm1[:])
            nc.vector.tensor_add(wv[:], zc[:], rc[:])
            nc.vector.tensor_scalar_mul(wv[:], wv[:], EM)          # W = e^{dc+m}

            # ---------------- phat row features (baseline layout) ---------
            phatnat = prep.tile([128, NBLK, 128], BF16)
            nc.gpsimd.memset(phatnat[:], 0.0)
            nc.vector.tensor_mul(tmp[:], alpha[:], pn[:])
            nc.vector.tensor_scalar_mul(phatnat[:, :, 64], tmp[:], 2.0)
            nc.vector.tensor_scalar_mul(phatnat[:, :, 65], alpha[:], 2.0)
            nc.vector.memset(phatnat[:, :, 66], 1.0)
            for j in range(NBLK):
                nc.gpsimd.tensor_scalar(
                    phatnat[:, j, 0:64], prednat[:, j, :],
                    alpha[:, ds(j, 1)], -4.0, ALU.mult, ALU.mult,
                )
                nc.sync.dma_start_transpose(
                    phatT[:, ts(j, 128)], phatnat[:, j, :]
                )

        # ---------------- phase A ----------------
        with (
            tc.tile_pool(name="mma", bufs=2, space=PSUM) as pa,
            tc.tile_pool(name="mmb", bufs=2, space=PSUM) as pb,
            tc.tile_pool(name="zsq", bufs=6) as zsqpool,
            tc.tile_pool(name="sw", bufs=6) as swpool,
            tc.tile_pool(name="ubuf", bufs=6) as upool,
            tc.tile_pool(name="p1", bufs=3) as p1pool,
            tc.tile_pool(name="p2", bufs=4) as p2pool,
        ):
            p1_tiles = []
            for i in range(4):
                t = p1pool.tile([128, P1W], BF16, name=f"p1_{i}", tag=f"p1_{i}")
                nc.vector.memset(t[:, 625:P1W], 1.0)
                p1_tiles.append(t)

            HALF = 625
            for j in range(NBLK):
                tpsA = pa.tile([128, HALF], F32, name="tpsA", tag="tpsA")
                tpsB = pb.tile([128, HALF], F32, name="tpsB", tag="tpsB")
                for t, base in ((tpsA, 0), (tpsB, HALF)):
                    for c0, cw in ((0, 512), (512, 113)):
                        nc.tensor.matmul(
                            t[:, ds(c0, cw)],
                            phatT[0:67, ts(j, 128)],
                            ahatT[:, ds(base + c0, cw)],
                            start=True,
                            stop=True,
                        )
                # Z = z^2 (- EPS_T on the DVE path)
                zsq = zsqpool.tile([128, CS], F32, name="zsq")
                # spread DVE squares: half A on DVE when j%3==2, half B when
                # j%3==1, so no single block doubles up DVE work
                dve_half = {0: None, 1: 1, 2: 0}[j % 3]
                for hi, (t, base) in enumerate(((tpsA, 0), (tpsB, HALF))):
                    if hi == dve_half:
                        nc.vector._custom_dve(
                            sq_bias, out=zsq[:, ds(base, HALF)], in0=t[:],
                            s0=EPS_T,
                        )
                    else:
                        nc.scalar.activation(
                            zsq[:, ds(base, HALF)], t[:], AF.Square
                        )
                # sbar = sqrt(z^2 - EPS_T); biased halves handled separately
                sw = swpool.tile([128, CS], F32, name="sw")
                if dve_half is None:
                    nc.scalar.activation(sw[:], zsq[:], AF.Sqrt, bias=biasme[:])
                else:
                    ob = HALF * (1 - dve_half)
                    db = HALF * dve_half
                    nc.scalar.activation(
                        sw[:, ds(ob, HALF)], zsq[:, ds(ob, HALF)], AF.Sqrt,
                        bias=biasme[:],
                    )
                    nc.scalar.activation(
                        sw[:, ds(db, HALF)], zsq[:, ds(db, HALF)], AF.Sqrt
                    )
                # u = max(W*(z - sbar), 1) = max(e^{g-d}, 1)
                u = upool.tile([128, CS], BF16, name="u")
                for t, base in ((tpsA, 0), (tpsB, HALF)):
                    nc.vector._custom_dve(
                        clamp_merge, out=u[:, ds(base, HALF)], in0=t[:],
                        in1=sw[:, ds(base, HALF)], s0=wv[:, ds(j, 1)],
                    )
                # pair products: [1250] -> [625] (+15 pad ones) -> 320 -> 160
                p1 = p1_tiles[j % 4]
                peng = nc.gpsimd if j % 2 == 0 else nc.vector
                peng.tensor_mul(p1[:, 0:625], u[:, 0:625], u[:, 625:1250])
                p2 = p2pool.tile([128, 320], BF16, name="p2")
                nc.gpsimd.tensor_mul(p2[:], p1[:, 0:320], p1[:, 320:640])
                nc.gpsimd.tensor_mul(
                    p3buf[:, j, :], p2[:, 0:160], p2[:, 160:320]
                )

            # ---------------- phase B: ln + row-sum accumulate ----------
            # each ln spans ALL blocks (inner-col slice) so phase B only
            # starts after the last block -> no sqrt/ln table thrash
            for k in range(NGRP):
                lt = zsqpool.tile([128, NBLK, LNW], BF16, tag="lnout")
                nc.scalar.activation(
                    lt[:], p3buf[:, :, ds(k * LNW, LNW)], AF.Ln,
                    accum_out=hacc[:, ds(k, 1)],
                )

            # ---------------- final: per-partition sums to host ----------
            hsum = persist.tile([128, 1], F32)
            nc.vector.tensor_reduce(hsum[:], hacc[:], mybir.AxisListType.X, ALU.add)
            nc.sync.dma_start(out_d[:], hsum[:])


def _get_nc():
    if "nc" not in _CACHE:
        _CACHE["nc"] = _build()
    return _CACHE["nc"]


def kernel(pred_embs, target_embs, all_embs):
    pred = np.ascontiguousarray(np.asarray(pred_embs, dtype=np.float32))
    targ = np.ascontiguousarray(np.asarray(target_embs, dtype=np.float32))
    alls = np.ascontiguousarray(np.asarray(all_embs, dtype=np.float32))

    nc = _get_nc()
    in_maps = [
        {"pred": pred, "targ": targ, "alls": alls[c * CS:(c + 1) * CS]}
        for c in range(NCORES)
    ]
    res = run_bass_kernel_spmd(nc, in_maps, list(range(NCORES)))
    hinge = sum(float(r["partial"].sum()) for r in res.results)
    loss = (hinge - MARGIN * B) / B
    return np.float32(loss)


if __name__ == "__main__":
    rng = np.random.RandomState(0)

    def ball(rng, n):
        v = rng.randn(n, D).astype(np.float32)
        v /= np.linalg.norm(v, axis=1, keepdims=True) + 1e-8
        r = rng.rand(n, 1).astype(np.float32) * 0.9
        return v * r

    p = ball(rng, B)
    t = ball(rng, B)
    a = ball(rng, C)
    print(kernel(pred_embs=p, target_embs=t, all_embs=a))


# revision 3
# speedup vs baseline: 1.0233x; 1.0233x over previous
import sys

sys.path.insert(0, "/opt/trn_rl_repo")

import math

import numpy as np

import concourse.bacc as bacc
import concourse.bass as bass
import concourse.mybir as mybir
import concourse.tile as tile
from concourse.bass import ds, ts
from concourse.bass_utils import run_bass_kernel_spmd
from concourse.masks import make_identity

B, C, D = 4096, 10000, 64
NCORES = 8
CS = C // NCORES            # 1250 classes per core
NBLK = B // 128             # 32 row blocks of 128
PA = 125                    # partition rows per all_embs block
JA = CS // PA               # 10
MARGIN = 0.1
EM = math.exp(MARGIN)
EPS_T = 1.0 - 1e-5          # sqrt arg = tau0^2 - EPS_T*W^2; z~1 noise gives
                            # sqrt(<0)=NaN which the DVE max drops to u=1
NGRP = 8                    # phase-B ln instructions (each spans all blocks)
LNW = 160 // NGRP           # inner cols per ln instruction
P1W = 640                   # 512 + 113 merged pairs + 15 pad cols of 1.0
SUBP = ((0, 512), (512, 512), (1024, 226))   # psum sub-panels (1 bank each)

F32 = mybir.dt.float32
F32R = mybir.dt.float32r
BF16 = mybir.dt.bfloat16
AF = mybir.ActivationFunctionType
ALU = mybir.AluOpType
PSUM = bass.MemorySpace.PSUM

_CACHE = {}

# ---------------------------------------------------------------------------
# custom DVE ops
# ---------------------------------------------------------------------------


def _register_custom_ops():
    import concourse.dve_ops as dve_ops
    from concourse.dve_ops import OPS, CUSTOM_DVE_SPECS, DveOp
    from concourse.dve_spec import Spec, Src0, Src1, C0, One, maxx, sq, lower
    from concourse.dve_uop import DveOpSpec
    from concourse.dve_table_gen import dve_ver_for

    if "SUBCLAMP_W_PEH" in CUSTOM_DVE_SPECS:
        return (
            dve_ops._PEH_SQUARE_BIAS,  # type: ignore[attr-defined]
            dve_ops._PEH_CLAMP_MERGE,  # type: ignore[attr-defined]
        )

    def mk(name, spec, rd1):
        row = dve_ops._CUSTOM_DVE_ROW_BASE + len(OPS)
        shas = {}
        for ver in ("v3", "v4"):
            try:
                tmp = DveOpSpec(
                    name=name, opcode=row, uops=lower(spec, ver=ver), rd1_en=rd1
                )
                shas[ver] = tmp.sha(ver)
            except Exception:
                pass
        op = DveOp(name, spec, subdim=False, uops_sha=shas)
        OPS.append(op)
        CUSTOM_DVE_SPECS[name] = spec
        dve_ops._SUB_OPCODE_FOR_NAME[name] = row
        return op

    sq_bias = mk(
        "SQUARE_BIAS_PEH",
        Spec(
            body=sq(Src0) - C0,
            reference=lambda in0, in1, s0, s1, imm2: (
                in0.astype(np.float32) * in0.astype(np.float32) - s0
            ),
        ),
        rd1=False,
    )

    def _sc_ref(in0, in1, s0, s1, imm2):
        v = (in0.astype(np.float32) - in1.astype(np.float32)) * s0
        return np.maximum(np.nan_to_num(v, nan=0.0), 1.0)

    clamp_merge = mk(
        "SUBCLAMP_W_PEH",
        Spec(body=maxx((Src0 - Src1) * C0, One), reference=_sc_ref),
        rd1=True,
    )
    dve_ops._PEH_SQUARE_BIAS = sq_bias  # type: ignore[attr-defined]
    dve_ops._PEH_CLAMP_MERGE = clamp_merge  # type: ignore[attr-defined]
    return sq_bias, clamp_merge


def _patch_act_tables():
    """Pin Square/Sqrt to sqrt_and_others and Ln to natural_log by removing
    them from every other set. Only membership changes; list order (and so
    act_func_set_id) is untouched, and the pinned sets genuinely contain the
    functions on hardware."""
    if getattr(bacc, "_peh_act_patch", False):
        return
    orig = bacc.get_activation_tables

    def patched(arch):
        tabs = {k: set(v) for k, v in orig(arch).items()}
        for name, funcs in tabs.items():
            if name != "sqrt_and_others":
                funcs.discard(AF.Square)
                funcs.discard(AF.Sqrt)
            if name != "natural_log":
                funcs.discard(AF.Ln)
        return tabs

    bacc.get_activation_tables = patched
    bacc._peh_act_patch = True


# ---------------------------------------------------------------------------
# kernel body
# ---------------------------------------------------------------------------


def _build():
    sq_bias, clamp_merge = _register_custom_ops()
    _patch_act_tables()
    nc = bacc.Bacc(None, target_bir_lowering=False)
    pred_d = nc.declare_dram_parameter("pred", [B, D], F32, isOutput=False)
    targ_d = nc.declare_dram_parameter("targ", [B, D], F32, isOutput=False)
    alls_d = nc.declare_dram_parameter("alls", [CS, D], F32, isOutput=False)
    out_d = nc.declare_dram_parameter("partial", [128, 1], F32, isOutput=True)

    with tile.TileContext(nc) as tc:
        _body(nc, tc, pred_d, targ_d, alls_d, out_d, sq_bias, clamp_merge)
    nc.compile()
    return nc


def _body(nc, tc, pred_d, targ_d, alls_d, out_d, sq_bias, clamp_merge):
    with (
        tc.tile_pool(name="persist", bufs=1) as persist,
        tc.tile_pool(name="prep", bufs=1) as prep,
    ):
        # The 67-row augmented matmul gives z = cosh(d) per (row, class).
        # sbar = sqrt(z^2 - EPS_T) ~ sinh(d), and one fused DVE op computes
        # u = max(W*(z - sbar), 1) = max(e^{g-d}, 1), so each hinge term is
        # ln(u); products of u's shrink the final Ln pass 8x.
        phatT = persist.tile([128, B], BF16)
        ahatT = persist.tile([67, CS], BF16)
        hacc = persist.tile([128, NGRP], F32)
        ident = persist.tile([128, 128], BF16)
        p3buf = persist.tile([128, NBLK, 160], BF16)

        make_identity(nc, ident[:])
        biasm1 = persist.tile([128, 1], F32)
        nc.vector.memset(biasm1[:], -(1.0 - 2e-7))
        biasme = persist.tile([128, 1], F32)
        nc.vector.memset(biasme[:], -EPS_T)

        # ---------------- loads ----------------
        prednat = prep.tile([128, NBLK, D], F32)
        targnat = prep.tile([128, NBLK, D], F32)
        allnat = prep.tile([PA, JA, D], F32)
        nc.sync.dma_start(allnat[:], alls_d[:].rearrange("(j p) d -> p j d", p=PA))
        nc.sync.dma_start(prednat[:], pred_d[:].rearrange("(j p) d -> p j d", p=128))
        nc.scalar.dma_start(targnat[:], targ_d[:].rearrange("(j p) d -> p j d", p=128))

        with tc.tile_pool(name="prep_ps", bufs=2, space=PSUM) as prep_ps:
            # ---------------- ahat path (gates all matmuls) ----------------
            asq = prep.tile([PA, JA, D], F32)
            an = prep.tile([PA, JA], F32)
            beta = prep.tile([PA, JA], F32)
            tmpa = prep.tile([PA, JA], F32)
            nc.vector.tensor_mul(asq[:], allnat[:], allnat[:])
            nc.vector.tensor_reduce(an[:], asq[:], mybir.AxisListType.X, ALU.add)
            nc.vector.tensor_scalar(tmpa[:], an[:], -1.0, 1.0, ALU.mult, ALU.add)
            nc.vector.reciprocal(beta[:], tmpa[:])

            ahatnat = prep.tile([PA, JA, 67], BF16)
            for j in range(JA):
                nc.vector.tensor_scalar_mul(
                    ahatnat[:, j, 0:64], allnat[:, j, :], beta[:, ds(j, 1)]
                )
            nc.vector.tensor_copy(ahatnat[:, :, 64], beta[:])
            nc.vector.tensor_mul(tmpa[:], beta[:], an[:])
            nc.vector.tensor_copy(ahatnat[:, :, 65], tmpa[:])
            nc.vector.memset(ahatnat[:, :, 66], 1.0)
            for jj in range(JA):
                pt = prep_ps.tile([67, 128], BF16)
                nc.tensor.transpose(
                    pt[0:67, 0:PA], ahatnat[:, jj, :], ident[0:PA, 0:PA]
                )
                nc.vector.tensor_copy(ahatT[:, ts(jj, PA)], pt[0:67, 0:PA])

            # ---------------- pn/tn, alpha, W = e^{dc+m} ----------------
            sq = prep.tile([128, NBLK, D], F32)
            pn = prep.tile([128, NBLK], F32)
            tn = prep.tile([128, NBLK], F32)
            alpha = prep.tile([128, NBLK], F32)
            alphat = prep.tile([128, NBLK], F32)
            tmp = prep.tile([128, NBLK], F32)

            nc.vector.tensor_mul(sq[:], prednat[:], prednat[:])
            nc.vector.tensor_reduce(pn[:], sq[:], mybir.AxisListType.X, ALU.add)
            nc.vector.tensor_scalar(tmp[:], pn[:], -1.0, 1.0, ALU.mult, ALU.add)
            nc.vector.reciprocal(alpha[:], tmp[:])

            sqt = prep.tile([128, NBLK, D], F32, name="sqt")
            nc.gpsimd.tensor_mul(sqt[:], targnat[:], targnat[:])
            nc.vector.tensor_reduce(tn[:], sqt[:], mybir.AxisListType.X, ALU.add)
            nc.vector.tensor_scalar(tmp[:], tn[:], -1.0, 1.0, ALU.mult, ALU.add)
            nc.vector.reciprocal(alphat[:], tmp[:])

            s2c = prep.tile([128, NBLK], F32)
            zc = prep.tile([128, NBLK], F32)
            zzc = prep.tile([128, NBLK], F32)
            rc = prep.tile([128, NBLK], F32)
            wv = prep.tile([128, NBLK], F32)
            nc.vector.tensor_sub(sq[:], prednat[:], targnat[:])
            nc.vector.tensor_mul(sq[:], sq[:], sq[:])
            nc.vector.tensor_reduce(s2c[:], sq[:], mybir.AxisListType.X, ALU.add)
            nc.vector.tensor_mul(s2c[:], s2c[:], alpha[:])
            nc.vector.tensor_mul(s2c[:], s2c[:], alphat[:])
            nc.vector.tensor_scalar(zc[:], s2c[:], 2.0, 1.0, ALU.mult, ALU.add)
            nc.vector.tensor_mul(zzc[:], zc[:], zc[:])
            nc.scalar.activation(rc[:], zzc[:], AF.Sqrt, bias=biasm1[:])
            nc.vector.tensor_add(wv[:], zc[:], rc[:])
            nc.vector.tensor_scalar_mul(wv[:], wv[:], EM)          # W = e^{dc+m}

            # ---------------- phat row features (baseline layout) ---------
            phatnat = prep.tile([128, NBLK, 128], BF16)
            nc.gpsimd.memset(phatnat[:], 0.0)
            nc.vector.tensor_mul(tmp[:], alpha[:], pn[:])
            nc.vector.tensor_scalar_mul(phatnat[:, :, 64], tmp[:], 2.0)
            nc.vector.tensor_scalar_mul(phatnat[:, :, 65], alpha[:], 2.0)
            nc.vector.memset(phatnat[:, :, 66], 1.0)
            for j in range(NBLK):
                nc.gpsimd.tensor_scalar(
                    phatnat[:, j, 0:64], prednat[:, j, :],
                    alpha[:, ds(j, 1)], -4.0, ALU.mult, ALU.mult,
                )
                nc.sync.dma_start_transpose(
                    phatT[:, ts(j, 128)], phatnat[:, j, :]
                )

        # ---------------- phase A ----------------
        with (
            tc.tile_pool(name="mma", bufs=2, space=PSUM) as pa,
            tc.tile_pool(name="mmb", bufs=2, space=PSUM) as pb,
            tc.tile_pool(name="zsq", bufs=6) as zsqpool,
            tc.tile_pool(name="sw", bufs=6) as swpool,
            tc.tile_pool(name="ubuf", bufs=6) as upool,
            tc.tile_pool(name="p1", bufs=3) as p1pool,
            tc.tile_pool(name="p2", bufs=4) as p2pool,
        ):
            p1_tiles = []
            for i in range(4):
                t = p1pool.tile([128, P1W], BF16, name=f"p1_{i}", tag=f"p1_{i}")
                nc.vector.memset(t[:, 625:P1W], 1.0)
                p1_tiles.append(t)

            HALF = 625
            for j in range(NBLK):
                tpsA = pa.tile([128, HALF], F32, name="tpsA", tag="tpsA")
                tpsB = pb.tile([128, HALF], F32, name="tpsB", tag="tpsB")
                for t, base in ((tpsA, 0), (tpsB, HALF)):
                    for c0, cw in ((0, 512), (512, 113)):
                        nc.tensor.matmul(
                            t[:, ds(c0, cw)],
                            phatT[0:67, ts(j, 128)],
                            ahatT[:, ds(base + c0, cw)],
                            start=True,
                            stop=True,
                        )
                # Z = z^2 (- EPS_T on the DVE path)
                zsq = zsqpool.tile([128, CS], F32, name="zsq")
                # spread DVE squares: half A on DVE when j%3==2, half B when
                # j%3==1, so no single block doubles up DVE work
                dve_half = {0: None, 1: 1, 2: None, 3: 0}[j % 4]
                for hi, (t, base) in enumerate(((tpsA, 0), (tpsB, HALF))):
                    if hi == dve_half:
                        nc.vector._custom_dve(
                            sq_bias, out=zsq[:, ds(base, HALF)], in0=t[:],
                            s0=EPS_T,
                        )
                    else:
                        nc.scalar.activation(
                            zsq[:, ds(base, HALF)], t[:], AF.Square
                        )
                # sbar = sqrt(z^2 - EPS_T); biased halves handled separately
                sw = swpool.tile([128, CS], F32, name="sw")
                if dve_half is None:
                    nc.scalar.activation(sw[:], zsq[:], AF.Sqrt, bias=biasme[:])
                else:
                    ob = HALF * (1 - dve_half)
                    db = HALF * dve_half
                    nc.scalar.activation(
                        sw[:, ds(ob, HALF)], zsq[:, ds(ob, HALF)], AF.Sqrt,
                        bias=biasme[:],
                    )
                    nc.scalar.activation(
                        sw[:, ds(db, HALF)], zsq[:, ds(db, HALF)], AF.Sqrt
                    )
                # u = max(W*(z - sbar), 1) = max(e^{g-d}, 1)
                u = upool.tile([128, CS], BF16, name="u")
                for t, base in ((tpsA, 0), (tpsB, HALF)):
                    nc.vector._custom_dve(
                        clamp_merge, out=u[:, ds(base, HALF)], in0=t[:],
                        in1=sw[:, ds(base, HALF)], s0=wv[:, ds(j, 1)],
                    )
                # pair products: [1250] -> [625] (+15 pad ones) -> 320 -> 160
                p1 = p1_tiles[j % 4]
                peng = nc.gpsimd if j % 2 == 0 else nc.vector
                peng.tensor_mul(p1[:, 0:625], u[:, 0:625], u[:, 625:1250])
                p2 = p2pool.tile([128, 320], BF16, name="p2")
                nc.gpsimd.tensor_mul(p2[:], p1[:, 0:320], p1[:, 320:640])
                nc.gpsimd.tensor_mul(
                    p3buf[:, j, :], p2[:, 0:160], p2[:, 160:320]
                )

            # ---------------- phase B: ln + row-sum accumulate ----------
            # each ln spans ALL blocks (inner-col slice) so phase B only
            # starts after the last block -> no sqrt/ln table thrash
            for k in range(NGRP):
                lt = zsqpool.tile([128, NBLK, LNW], BF16, tag="lnout")
                nc.scalar.activation(
                    lt[:], p3buf[:, :, ds(k * LNW, LNW)], AF.Ln,
                    accum_out=hacc[:, ds(k, 1)],
                )

            # ---------------- final: per-partition sums to host ----------
            hsum = persist.tile([128, 1], F32)
            nc.vector.tensor_reduce(hsum[:], hacc[:], mybir.AxisListType.X, ALU.add)
            nc.sync.dma_start(out_d[:], hsum[:])


def _get_nc():
    if "nc" not in _CACHE:
        _CACHE["nc"] = _build()
    return _CACHE["nc"]


def kernel(pred_embs, target_embs, all_embs):
    pred = np.ascontiguousarray(np.asarray(pred_embs, dtype=np.float32))
    targ = np.ascontiguousarray(np.asarray(target_embs, dtype=np.float32))
    alls = np.ascontiguousarray(np.asarray(all_embs, dtype=np.float32))

    nc = _get_nc()
    in_maps = [
        {"pred": pred, "targ": targ, "alls": alls[c * CS:(c + 1) * CS]}
        for c in range(NCORES)
    ]
    res = run_bass_kernel_spmd(nc, in_maps, list(range(NCORES)))
    hinge = sum(float(r["partial"].sum()) for r in res.results)
    loss = (hinge - MARGIN * B) / B
    return np.float32(loss)


if __name__ == "__main__":
    rng = np.random.RandomState(0)

    def ball(rng, n):
        v = rng.randn(n, D).astype(np.float32)
        v /= np.linalg.norm(v, axis=1, keepdims=True) + 1e-8
        r = rng.rand(n, 1).astype(np.float32) * 0.9
        return v * r

    p = ball(rng, B)
    t = ball(rng, B)
    a = ball(rng, C)
    print(kernel(pred_embs=p, target_embs=t, all_embs=a))


# revision 16
# speedup vs baseline: 1.1110x; 1.0857x over previous
import sys

sys.path.insert(0, "/opt/trn_rl_repo")

import math

import numpy as np

import concourse.bacc as bacc
import concourse.bass as bass
import concourse.mybir as mybir
import concourse.tile as tile
from concourse.bass import ds, ts
from concourse.bass_utils import run_bass_kernel_spmd
from concourse.masks import make_identity

B, C, D = 4096, 10000, 64
NCORES = 8
CS = C // NCORES            # 1250 classes per core
NBLK = B // 128             # 32 row blocks of 128
PA = 125                    # partition rows per all_embs block
JA = CS // PA               # 10
MARGIN = 0.1
EM = math.exp(MARGIN)
EPS_T = 1.0 - 1e-5          # sqrt arg = tau0^2 - EPS_T*W^2; z~1 noise gives
                            # sqrt(<0)=NaN which the DVE max drops to u=1
NGRP = 8                    # phase-B ln instructions (each spans all blocks)
LNW = 160 // NGRP           # inner cols per ln instruction
P1W = 640                   # 512 + 113 merged pairs + 15 pad cols of 1.0
SUBP = ((0, 512), (512, 512), (1024, 226))   # psum sub-panels (1 bank each)

F32 = mybir.dt.float32
F32R = mybir.dt.float32r
BF16 = mybir.dt.bfloat16
AF = mybir.ActivationFunctionType
ALU = mybir.AluOpType
PSUM = bass.MemorySpace.PSUM

_CACHE = {}

# ---------------------------------------------------------------------------
# custom DVE ops
# ---------------------------------------------------------------------------


def _register_custom_ops():
    import concourse.dve_ops as dve_ops
    from concourse.dve_ops import OPS, CUSTOM_DVE_SPECS, DveOp
    from concourse.dve_spec import Spec, Src0, Src1, C0, One, maxx, sq, lower
    from concourse.dve_uop import DveOpSpec
    from concourse.dve_table_gen import dve_ver_for

    if "SUBCLAMP_W_PEH" in CUSTOM_DVE_SPECS:
        return (
            dve_ops._PEH_SQUARE_BIAS,  # type: ignore[attr-defined]
            dve_ops._PEH_CLAMP_MERGE,  # type: ignore[attr-defined]
        )

    def mk(name, spec, rd1):
        row = dve_ops._CUSTOM_DVE_ROW_BASE + len(OPS)
        shas = {}
        for ver in ("v3", "v4"):
            try:
                tmp = DveOpSpec(
                    name=name, opcode=row, uops=lower(spec, ver=ver), rd1_en=rd1
                )
                shas[ver] = tmp.sha(ver)
            except Exception:
                pass
        op = DveOp(name, spec, subdim=False, uops_sha=shas)
        OPS.append(op)
        CUSTOM_DVE_SPECS[name] = spec
        dve_ops._SUB_OPCODE_FOR_NAME[name] = row
        return op

    sq_bias = mk(
        "SQUARE_BIAS_PEH",
        Spec(
            body=sq(Src0) - C0,
            reference=lambda in0, in1, s0, s1, imm2: (
                in0.astype(np.float32) * in0.astype(np.float32) - s0
            ),
        ),
        rd1=False,
    )

    def _sc_ref(in0, in1, s0, s1, imm2):
        v = (in0.astype(np.float32) - in1.astype(np.float32)) * s0
        return np.maximum(np.nan_to_num(v, nan=0.0), 1.0)

    clamp_merge = mk(
        "SUBCLAMP_W_PEH",
        Spec(body=maxx((Src0 - Src1) * C0, One), reference=_sc_ref),
        rd1=True,
    )
    dve_ops._PEH_SQUARE_BIAS = sq_bias  # type: ignore[attr-defined]
    dve_ops._PEH_CLAMP_MERGE = clamp_merge  # type: ignore[attr-defined]
    return sq_bias, clamp_merge


def _patch_act_tables():
    """Pin Square/Sqrt to sqrt_and_others and Ln to natural_log by removing
    them from every other set. Only membership changes; list order (and so
    act_func_set_id) is untouched, and the pinned sets genuinely contain the
    functions on hardware."""
    if getattr(bacc, "_peh_act_patch", False):
        return
    orig = bacc.get_activation_tables

    def patched(arch):
        tabs = {k: set(v) for k, v in orig(arch).items()}
        for name, funcs in tabs.items():
            if name != "sqrt_and_others":
                funcs.discard(AF.Square)
                funcs.discard(AF.Sqrt)
            if name != "natural_log":
                funcs.discard(AF.Ln)
        return tabs

    bacc.get_activation_tables = patched
    bacc._peh_act_patch = True


# ---------------------------------------------------------------------------
# kernel body
# ---------------------------------------------------------------------------


def _build():
    sq_bias, clamp_merge = _register_custom_ops()
    _patch_act_tables()
    nc = bacc.Bacc(None, target_bir_lowering=False)
    pred_d = nc.declare_dram_parameter("pred", [B, D], F32, isOutput=False)
    targ_d = nc.declare_dram_parameter("targ", [B, D], F32, isOutput=False)
    alls_d = nc.declare_dram_parameter("alls", [CS, D], F32, isOutput=False)
    out_d = nc.declare_dram_parameter("partial", [128, 1], F32, isOutput=True)

    with tile.TileContext(nc) as tc:
        _body(nc, tc, pred_d, targ_d, alls_d, out_d, sq_bias, clamp_merge)
    nc.compile()
    return nc


def _body(nc, tc, pred_d, targ_d, alls_d, out_d, sq_bias, clamp_merge):
    with (
        tc.tile_pool(name="persist", bufs=1) as persist,
        tc.tile_pool(name="prep", bufs=1) as prep,
    ):
        # The 67-row augmented matmul gives z = cosh(d) per (row, class).
        # sbar = sqrt(z^2 - EPS_T) ~ sinh(d), and one fused DVE op computes
        # u = max(W*(z - sbar), 1) = max(e^{g-d}, 1), so each hinge term is
        # ln(u); products of u's shrink the final Ln pass 8x.
        phatT = persist.tile([128, B], BF16)
        ahatT = persist.tile([67, CS], BF16)
        hacc = persist.tile([128, NGRP], F32)
        ident = persist.tile([128, 128], BF16)
        p3buf = persist.tile([128, NBLK, 80], BF16)

        make_identity(nc, ident[:])
        biasm1 = persist.tile([128, 1], F32)
        nc.vector.memset(biasm1[:], -(1.0 - 2e-7))
        biasme = persist.tile([128, 1], F32)
        nc.vector.memset(biasme[:], -EPS_T)

        # ---------------- loads ----------------
        prednat = prep.tile([128, NBLK, D], F32)
        targnat = prep.tile([128, NBLK, D], F32)
        allnat = prep.tile([PA, JA, D], F32)
        nc.sync.dma_start(allnat[:], alls_d[:].rearrange("(j p) d -> p j d", p=PA))
        PCHUNKS = ((0, 8), (8, 24))
        for c0, cn in PCHUNKS:
            nc.sync.dma_start(
                prednat[:, ds(c0, cn), :],
                pred_d[ds(c0 * 128, cn * 128), :].rearrange(
                    "(j p) d -> p j d", p=128
                ),
            )
            nc.scalar.dma_start(
                targnat[:, ds(c0, cn), :],
                targ_d[ds(c0 * 128, cn * 128), :].rearrange(
                    "(j p) d -> p j d", p=128
                ),
            )

        with tc.tile_pool(name="prep_ps", bufs=2, space=PSUM) as prep_ps:
            # ---------------- ahat path (gates all matmuls) ----------------
            asq = prep.tile([PA, JA, D], F32)
            an = prep.tile([PA, JA], F32)
            beta = prep.tile([PA, JA], F32)
            tmpa = prep.tile([PA, JA], F32)
            nc.vector.tensor_mul(asq[:], allnat[:], allnat[:])
            nc.vector.tensor_reduce(an[:], asq[:], mybir.AxisListType.X, ALU.add)
            nc.vector.tensor_scalar(tmpa[:], an[:], -1.0, 1.0, ALU.mult, ALU.add)
            nc.vector.reciprocal(beta[:], tmpa[:])

            ahatnat = prep.tile([PA, JA, 67], BF16)
            for j in range(JA):
                nc.vector.tensor_scalar_mul(
                    ahatnat[:, j, 0:64], allnat[:, j, :], beta[:, ds(j, 1)]
                )
            nc.vector.tensor_copy(ahatnat[:, :, 64], beta[:])
            nc.vector.tensor_mul(tmpa[:], beta[:], an[:])
            nc.vector.tensor_copy(ahatnat[:, :, 65], tmpa[:])
            nc.vector.memset(ahatnat[:, :, 66], 1.0)
            for jj in range(JA):
                pt = prep_ps.tile([67, 128], BF16)
                nc.tensor.transpose(
                    pt[0:67, 0:PA], ahatnat[:, jj, :], ident[0:PA, 0:PA]
                )
                nc.vector.tensor_copy(ahatT[:, ts(jj, PA)], pt[0:67, 0:PA])

            # ---------------- pn/tn, alpha, W = e^{dc+m} ----------------
            sq = prep.tile([128, NBLK, D], F32)
            pn = prep.tile([128, NBLK], F32)
            tn = prep.tile([128, NBLK], F32)
            alpha = prep.tile([128, NBLK], F32)
            alphat = prep.tile([128, NBLK], F32)
            tmp = prep.tile([128, NBLK], F32)



            sqt = prep.tile([128, NBLK, D], F32, name="sqt")
            s2c = prep.tile([128, NBLK], F32)
            zc = prep.tile([128, NBLK], F32)
            zzc = prep.tile([128, NBLK], F32)
            rc = prep.tile([128, NBLK], F32)
            wv = prep.tile([128, NBLK], F32)
            phatnat = prep.tile([128, NBLK, 128], BF16)
            nc.gpsimd.memset(phatnat[:], 0.0)
            for c0, cn in PCHUNKS:
                hs = ds(c0, cn)
                nc.vector.tensor_mul(
                    sq[:, hs, :], prednat[:, hs, :], prednat[:, hs, :]
                )
                nc.vector.tensor_reduce(
                    pn[:, hs], sq[:, hs, :], mybir.AxisListType.X, ALU.add
                )
                nc.gpsimd.tensor_mul(
                    sqt[:, hs, :], targnat[:, hs, :], targnat[:, hs, :]
                )
                nc.vector.tensor_reduce(
                    tn[:, hs], sqt[:, hs, :], mybir.AxisListType.X, ALU.add
                )
                nc.vector.tensor_scalar(
                    tmp[:, hs], pn[:, hs], -1.0, 1.0, ALU.mult, ALU.add
                )
                nc.vector.reciprocal(alpha[:, hs], tmp[:, hs])
                nc.vector.tensor_scalar(
                    tmp[:, hs], tn[:, hs], -1.0, 1.0, ALU.mult, ALU.add
                )
                nc.vector.reciprocal(alphat[:, hs], tmp[:, hs])
                nc.vector.tensor_sub(
                    sq[:, hs, :], prednat[:, hs, :], targnat[:, hs, :]
                )
                nc.vector.tensor_mul(sq[:, hs, :], sq[:, hs, :], sq[:, hs, :])
                nc.vector.tensor_reduce(
                    s2c[:, hs], sq[:, hs, :], mybir.AxisListType.X, ALU.add
                )
                nc.vector.tensor_mul(s2c[:, hs], s2c[:, hs], alpha[:, hs])
                nc.vector.tensor_mul(s2c[:, hs], s2c[:, hs], alphat[:, hs])
                nc.vector.tensor_scalar(
                    zc[:, hs], s2c[:, hs], 2.0, 1.0, ALU.mult, ALU.add
                )
                nc.vector.tensor_mul(zzc[:, hs], zc[:, hs], zc[:, hs])
                nc.scalar.activation(
                    rc[:, hs], zzc[:, hs], AF.Sqrt, bias=biasm1[:]
                )
                nc.vector.tensor_add(wv[:, hs], zc[:, hs], rc[:, hs])
                nc.vector.tensor_scalar_mul(wv[:, hs], wv[:, hs], EM)
                # phat features + transposes for this half's blocks
                nc.vector.tensor_mul(tmp[:, hs], alpha[:, hs], pn[:, hs])
                nc.vector.tensor_scalar_mul(phatnat[:, hs, 64], tmp[:, hs], 2.0)
                nc.vector.tensor_scalar_mul(
                    phatnat[:, hs, 65], alpha[:, hs], 2.0
                )
                nc.vector.memset(phatnat[:, hs, 66], 1.0)
                for j in range(c0, c0 + cn):
                    nc.gpsimd.tensor_scalar(
                        phatnat[:, j, 0:64], prednat[:, j, :],
                        alpha[:, ds(j, 1)], -4.0, ALU.mult, ALU.mult,
                    )
                    nc.sync.dma_start_transpose(
                        phatT[:, ts(j, 128)], phatnat[:, j, :]
                    )

        # ---------------- phase A ----------------
        with (
            tc.tile_pool(name="mma", bufs=2, space=PSUM) as pa,
            tc.tile_pool(name="mmb", bufs=2, space=PSUM) as pb,
            tc.tile_pool(name="zsq", bufs=6) as zsqpool,
            tc.tile_pool(name="sw", bufs=6) as swpool,
            tc.tile_pool(name="ubuf", bufs=6) as upool,
            tc.tile_pool(name="p1", bufs=3) as p1pool,
            tc.tile_pool(name="p2", bufs=4) as p2pool,
        ):
            p1_tiles = []
            for i in range(4):
                t = p1pool.tile([128, P1W], BF16, name=f"p1_{i}", tag=f"p1_{i}")
                nc.vector.memset(t[:, 625:P1W], 1.0)
                p1_tiles.append(t)

            HALF = 625
            for j in range(NBLK):
                tpsA = pa.tile([128, HALF], F32, name="tpsA", tag="tpsA")
                tpsB = pb.tile([128, HALF], F32, name="tpsB", tag="tpsB")
                for t, base in ((tpsA, 0), (tpsB, HALF)):
                    for c0, cw in ((0, 512), (512, 113)):
                        nc.tensor.matmul(
                            t[:, ds(c0, cw)],
                            phatT[0:67, ts(j, 128)],
                            ahatT[:, ds(base + c0, cw)],
                            start=True,
                            stop=True,
                        )
                # Z = z^2 (- EPS_T on the DVE path)
                zsq = zsqpool.tile([128, CS], F32, name="zsq")
                # spread DVE squares so no single block doubles up DVE work
                dve_half = {0: None, 1: 1, 2: None, 3: 0}[j % 4]
                for hi, (t, base) in enumerate(((tpsA, 0), (tpsB, HALF))):
                    if hi == dve_half:
                        nc.vector._custom_dve(
                            sq_bias, out=zsq[:, ds(base, HALF)], in0=t[:],
                            s0=0.0,
                        )
                    else:
                        nc.scalar.activation(
                            zsq[:, ds(base, HALF)], t[:], AF.Square
                        )
                # sbar = sqrt(z^2 - EPS_T)
                sw = swpool.tile([128, CS], F32, name="sw")
                nc.scalar.activation(sw[:], zsq[:], AF.Sqrt, bias=biasme[:])
                # u = max(W*(z - sbar), 1) = max(e^{g-d}, 1)
                u = upool.tile([128, CS], BF16, name="u")
                for t, base in ((tpsA, 0), (tpsB, HALF)):
                    nc.vector._custom_dve(
                        clamp_merge, out=u[:, ds(base, HALF)], in0=t[:],
                        in1=sw[:, ds(base, HALF)], s0=wv[:, ds(j, 1)],
                    )
                # pair products: [1250] -> [625] (+15 pad ones) -> 320 -> 160
                p1 = p1_tiles[j % 4]
                peng = nc.gpsimd if j % 2 == 0 else nc.vector
                peng.tensor_mul(p1[:, 0:625], u[:, 0:625], u[:, 625:1250])
                p2 = p2pool.tile([128, 320], BF16, name="p2")
                nc.gpsimd.tensor_mul(p2[:], p1[:, 0:320], p1[:, 320:640])
                p3 = p2pool.tile([128, 160], BF16, name="p3", tag="p3")
                nc.gpsimd.tensor_mul(p3[:], p2[:, 0:160], p2[:, 160:320])
                nc.gpsimd.tensor_mul(
                    p3buf[:, j, :], p3[:, 0:80], p3[:, 80:160]
                )

            # ---------------- phase B: ln + row-sum accumulate ----------
            # single fused ln spanning ALL blocks -> runs once after the
            # last block; accum_out gives the per-partition hinge sum
            lt = zsqpool.tile([128, NBLK, 80], BF16, name="lnout", tag="lnout")
            nc.scalar.activation(
                lt[:], p3buf[:], AF.Ln, accum_out=hacc[:, ds(0, 1)]
            )

            # ---------------- final: per-partition sums to host ----------
            nc.sync.dma_start(out_d[:], hacc[:, ds(0, 1)])


def _get_nc():
    if "nc" not in _CACHE:
        _CACHE["nc"] = _build()
    return _CACHE["nc"]


def kernel(pred_embs, target_embs, all_embs):
    pred = np.ascontiguousarray(np.asarray(pred_embs, dtype=np.float32))
    targ = np.ascontiguousarray(np.asarray(target_embs, dtype=np.float32))
    alls = np.ascontiguousarray(np.asarray(all_embs, dtype=np.float32))

    nc = _get_nc()
    in_maps = [
        {"pred": pred, "targ": targ, "alls": alls[c * CS:(c + 1) * CS]}
        for c in range(NCORES)
    ]
    res = run_bass_kernel_spmd(nc, in_maps, list(range(NCORES)))
    hinge = sum(float(r["partial"].sum()) for r in res.results)
    loss = (hinge - MARGIN * B) / B
    return np.float32(loss)


if __name__ == "__main__":
    rng = np.random.RandomState(0)

    def ball(rng, n):
        v = rng.randn(n, D).astype(np.float32)
        v /= np.linalg.norm(v, axis=1, keepdims=True) + 1e-8
        r = rng.rand(n, 1).astype(np.float32) * 0.9
        return v * r

    p = ball(rng, B)
    t = ball(rng, B)
    a = ball(rng, C)
    print(kernel(pred_embs=p, target_embs=t, all_embs=a))


# revision 23
# speedup vs baseline: 1.1128x; 1.0016x over previous
import sys

sys.path.insert(0, "/opt/trn_rl_repo")

import math

import numpy as np

import concourse.bacc as bacc
import concourse.bass as bass
import concourse.mybir as mybir
import concourse.tile as tile
from concourse.bass import ds, ts
from concourse.bass_utils import run_bass_kernel_spmd
from concourse.masks import make_identity

B, C, D = 4096, 10000, 64
NCORES = 8
CS = C // NCORES            # 1250 classes per core
NBLK = B // 128             # 32 row blocks of 128
PA = 125                    # partition rows per all_embs block
JA = CS // PA               # 10
MARGIN = 0.1
EM = math.exp(MARGIN)
EPS_T = 1.0 - 1e-5          # sqrt arg = tau0^2 - EPS_T*W^2; z~1 noise gives
                            # sqrt(<0)=NaN which the DVE max drops to u=1
NGRP = 8                    # phase-B ln instructions (each spans all blocks)
LNW = 160 // NGRP           # inner cols per ln instruction
P1W = 640                   # 512 + 113 merged pairs + 15 pad cols of 1.0
SUBP = ((0, 512), (512, 512), (1024, 226))   # psum sub-panels (1 bank each)

F32 = mybir.dt.float32
F32R = mybir.dt.float32r
BF16 = mybir.dt.bfloat16
AF = mybir.ActivationFunctionType
ALU = mybir.AluOpType
PSUM = bass.MemorySpace.PSUM

_CACHE = {}

# ---------------------------------------------------------------------------
# custom DVE ops
# ---------------------------------------------------------------------------


def _register_custom_ops():
    import concourse.dve_ops as dve_ops
    from concourse.dve_ops import OPS, CUSTOM_DVE_SPECS, DveOp
    from concourse.dve_spec import Spec, Src0, Src1, C0, One, maxx, sq, lower
    from concourse.dve_uop import DveOpSpec
    from concourse.dve_table_gen import dve_ver_for

    if "SUBCLAMP_W_PEH" in CUSTOM_DVE_SPECS:
        return (
            dve_ops._PEH_SQUARE_BIAS,  # type: ignore[attr-defined]
            dve_ops._PEH_CLAMP_MERGE,  # type: ignore[attr-defined]
        )

    def mk(name, spec, rd1):
        row = dve_ops._CUSTOM_DVE_ROW_BASE + len(OPS)
        shas = {}
        for ver in ("v3", "v4"):
            try:
                tmp = DveOpSpec(
                    name=name, opcode=row, uops=lower(spec, ver=ver), rd1_en=rd1
                )
                shas[ver] = tmp.sha(ver)
            except Exception:
                pass
        op = DveOp(name, spec, subdim=False, uops_sha=shas)
        OPS.append(op)
        CUSTOM_DVE_SPECS[name] = spec
        dve_ops._SUB_OPCODE_FOR_NAME[name] = row
        return op

    sq_bias = mk(
        "SQUARE_BIAS_PEH",
        Spec(
            body=sq(Src0) - C0,
            reference=lambda in0, in1, s0, s1, imm2: (
                in0.astype(np.float32) * in0.astype(np.float32) - s0
            ),
        ),
        rd1=False,
    )

    def _sc_ref(in0, in1, s0, s1, imm2):
        v = (in0.astype(np.float32) - in1.astype(np.float32)) * s0
        return np.maximum(np.nan_to_num(v, nan=0.0), 1.0)

    clamp_merge = mk(
        "SUBCLAMP_W_PEH",
        Spec(body=maxx((Src0 - Src1) * C0, One), reference=_sc_ref),
        rd1=True,
    )
    dve_ops._PEH_SQUARE_BIAS = sq_bias  # type: ignore[attr-defined]
    dve_ops._PEH_CLAMP_MERGE = clamp_merge  # type: ignore[attr-defined]
    return sq_bias, clamp_merge


def _patch_act_tables():
    """Pin Square/Sqrt to sqrt_and_others and Ln to natural_log by removing
    them from every other set. Only membership changes; list order (and so
    act_func_set_id) is untouched, and the pinned sets genuinely contain the
    functions on hardware."""
    if getattr(bacc, "_peh_act_patch", False):
        return
    orig = bacc.get_activation_tables

    def patched(arch):
        tabs = {k: set(v) for k, v in orig(arch).items()}
        for name, funcs in tabs.items():
            if name != "sqrt_and_others":
                funcs.discard(AF.Square)
                funcs.discard(AF.Sqrt)
            if name != "natural_log":
                funcs.discard(AF.Ln)
        return tabs

    bacc.get_activation_tables = patched
    bacc._peh_act_patch = True


# ---------------------------------------------------------------------------
# kernel body
# ---------------------------------------------------------------------------


def _build():
    sq_bias, clamp_merge = _register_custom_ops()
    _patch_act_tables()
    nc = bacc.Bacc(None, target_bir_lowering=False)
    pred_d = nc.declare_dram_parameter("pred", [B, D], F32, isOutput=False)
    targ_d = nc.declare_dram_parameter("targ", [B, D], F32, isOutput=False)
    alls_d = nc.declare_dram_parameter("alls", [CS, D], F32, isOutput=False)
    out_d = nc.declare_dram_parameter("partial", [128, 1], F32, isOutput=True)

    with tile.TileContext(nc) as tc:
        _body(nc, tc, pred_d, targ_d, alls_d, out_d, sq_bias, clamp_merge)
    nc.compile()
    return nc


def _body(nc, tc, pred_d, targ_d, alls_d, out_d, sq_bias, clamp_merge):
    with (
        tc.tile_pool(name="persist", bufs=1) as persist,
        tc.tile_pool(name="prep", bufs=1) as prep,
    ):
        # The 67-row augmented matmul gives z = cosh(d) per (row, class).
        # sbar = sqrt(z^2 - EPS_T) ~ sinh(d), and one fused DVE op computes
        # u = max(W*(z - sbar), 1) = max(e^{g-d}, 1), so each hinge term is
        # ln(u); products of u's shrink the final Ln pass 8x.
        phatT = persist.tile([128, B], BF16)
        ahatT = persist.tile([67, CS], BF16)
        hacc = persist.tile([128, NGRP], F32)
        ident = persist.tile([128, 128], BF16)
        p3buf = persist.tile([128, NBLK, 80], BF16)

        make_identity(nc, ident[:])
        biasm1 = persist.tile([128, 1], F32)
        nc.vector.memset(biasm1[:], -(1.0 - 2e-7))
        biasme = persist.tile([128, 1], F32)
        nc.vector.memset(biasme[:], -EPS_T)

        # ---------------- loads ----------------
        prednat = prep.tile([128, NBLK, D], F32)
        targnat = prep.tile([128, NBLK, D], F32)
        allnat = prep.tile([PA, JA, D], F32)
        nc.sync.dma_start(allnat[:], alls_d[:].rearrange("(j p) d -> p j d", p=PA))
        PCHUNKS = ((0, 12), (12, 20))
        for c0, cn in PCHUNKS:
            nc.sync.dma_start(
                prednat[:, ds(c0, cn), :],
                pred_d[ds(c0 * 128, cn * 128), :].rearrange(
                    "(j p) d -> p j d", p=128
                ),
            )
            nc.scalar.dma_start(
                targnat[:, ds(c0, cn), :],
                targ_d[ds(c0 * 128, cn * 128), :].rearrange(
                    "(j p) d -> p j d", p=128
                ),
            )

        with tc.tile_pool(name="prep_ps", bufs=2, space=PSUM) as prep_ps:
            # ---------------- ahat path (gates all matmuls) ----------------
            asq = prep.tile([PA, JA, D], F32)
            an = prep.tile([PA, JA], F32)
            beta = prep.tile([PA, JA], F32)
            tmpa = prep.tile([PA, JA], F32)
            nc.vector.tensor_mul(asq[:], allnat[:], allnat[:])
            nc.vector.tensor_reduce(an[:], asq[:], mybir.AxisListType.X, ALU.add)
            nc.vector.tensor_scalar(tmpa[:], an[:], -1.0, 1.0, ALU.mult, ALU.add)
            nc.vector.reciprocal(beta[:], tmpa[:])

            ahatnat = prep.tile([PA, JA, 67], BF16)
            for j in range(JA):
                nc.vector.tensor_scalar_mul(
                    ahatnat[:, j, 0:64], allnat[:, j, :], beta[:, ds(j, 1)]
                )
            nc.vector.tensor_copy(ahatnat[:, :, 64], beta[:])
            nc.vector.tensor_mul(tmpa[:], beta[:], an[:])
            nc.vector.tensor_copy(ahatnat[:, :, 65], tmpa[:])
            nc.vector.memset(ahatnat[:, :, 66], 1.0)
            for jj in range(JA):
                pt = prep_ps.tile([67, 128], BF16)
                nc.tensor.transpose(
                    pt[0:67, 0:PA], ahatnat[:, jj, :], ident[0:PA, 0:PA]
                )
                nc.vector.tensor_copy(ahatT[:, ts(jj, PA)], pt[0:67, 0:PA])

            # ---------------- pn/tn, alpha, W = e^{dc+m} ----------------
            sq = prep.tile([128, NBLK, D], F32)
            pn = prep.tile([128, NBLK], F32)
            tn = prep.tile([128, NBLK], F32)
            alpha = prep.tile([128, NBLK], F32)
            alphat = prep.tile([128, NBLK], F32)
            tmp = prep.tile([128, NBLK], F32)



            sqt = prep.tile([128, NBLK, D], F32, name="sqt")
            s2c = prep.tile([128, NBLK], F32)
            zc = prep.tile([128, NBLK], F32)
            zzc = prep.tile([128, NBLK], F32)
            rc = prep.tile([128, NBLK], F32)
            wv = prep.tile([128, NBLK], F32)
            phatnat = prep.tile([128, NBLK, 128], BF16)
            nc.gpsimd.memset(phatnat[:], 0.0)
            for c0, cn in PCHUNKS:
                hs = ds(c0, cn)
                nc.vector.tensor_mul(
                    sq[:, hs, :], prednat[:, hs, :], prednat[:, hs, :]
                )
                nc.vector.tensor_reduce(
                    pn[:, hs], sq[:, hs, :], mybir.AxisListType.X, ALU.add
                )
                nc.gpsimd.tensor_mul(
                    sqt[:, hs, :], targnat[:, hs, :], targnat[:, hs, :]
                )
                nc.vector.tensor_reduce(
                    tn[:, hs], sqt[:, hs, :], mybir.AxisListType.X, ALU.add
                )
                nc.vector.tensor_scalar(
                    tmp[:, hs], pn[:, hs], -1.0, 1.0, ALU.mult, ALU.add
                )
                nc.vector.reciprocal(alpha[:, hs], tmp[:, hs])
                nc.vector.tensor_scalar(
                    tmp[:, hs], tn[:, hs], -1.0, 1.0, ALU.mult, ALU.add
                )
                nc.vector.reciprocal(alphat[:, hs], tmp[:, hs])
                nc.vector.tensor_sub(
                    sq[:, hs, :], prednat[:, hs, :], targnat[:, hs, :]
                )
                nc.vector.tensor_mul(sq[:, hs, :], sq[:, hs, :], sq[:, hs, :])
                nc.vector.tensor_reduce(
                    s2c[:, hs], sq[:, hs, :], mybir.AxisListType.X, ALU.add
                )
                nc.vector.tensor_mul(s2c[:, hs], s2c[:, hs], alpha[:, hs])
                nc.vector.tensor_mul(s2c[:, hs], s2c[:, hs], alphat[:, hs])
                nc.vector.tensor_scalar(
                    zc[:, hs], s2c[:, hs], 2.0, 1.0, ALU.mult, ALU.add
                )
                nc.vector.tensor_mul(zzc[:, hs], zc[:, hs], zc[:, hs])
                nc.scalar.activation(
                    rc[:, hs], zzc[:, hs], AF.Sqrt, bias=biasm1[:]
                )
                nc.vector.tensor_add(wv[:, hs], zc[:, hs], rc[:, hs])
                nc.vector.tensor_scalar_mul(wv[:, hs], wv[:, hs], EM)
                # phat features + transposes for this half's blocks
                nc.vector.tensor_mul(tmp[:, hs], alpha[:, hs], pn[:, hs])
                nc.vector.tensor_scalar_mul(phatnat[:, hs, 64], tmp[:, hs], 2.0)
                nc.vector.tensor_scalar_mul(
                    phatnat[:, hs, 65], alpha[:, hs], 2.0
                )
                nc.vector.memset(phatnat[:, hs, 66], 1.0)
                for j in range(c0, c0 + cn):
                    nc.gpsimd.tensor_scalar(
                        phatnat[:, j, 0:64], prednat[:, j, :],
                        alpha[:, ds(j, 1)], -4.0, ALU.mult, ALU.mult,
                    )
                    nc.sync.dma_start_transpose(
                        phatT[:, ts(j, 128)], phatnat[:, j, :]
                    )

        # ---------------- phase A ----------------
        with (
            tc.tile_pool(name="mma", bufs=2, space=PSUM) as pa,
            tc.tile_pool(name="mmb", bufs=2, space=PSUM) as pb,
            tc.tile_pool(name="zsq", bufs=6) as zsqpool,
            tc.tile_pool(name="sw", bufs=6) as swpool,
            tc.tile_pool(name="ubuf", bufs=6) as upool,
            tc.tile_pool(name="p1", bufs=3) as p1pool,
            tc.tile_pool(name="p2", bufs=4) as p2pool,
        ):
            p1_tiles = []
            for i in range(4):
                t = p1pool.tile([128, P1W], BF16, name=f"p1_{i}", tag=f"p1_{i}")
                nc.vector.memset(t[:, 625:P1W], 1.0)
                p1_tiles.append(t)

            HALF = 625
            for j in range(NBLK):
                tpsA = pa.tile([128, HALF], F32, name="tpsA", tag="tpsA")
                tpsB = pb.tile([128, HALF], F32, name="tpsB", tag="tpsB")
                for t, base in ((tpsA, 0), (tpsB, HALF)):
                    for c0, cw in ((0, 512), (512, 113)):
                        nc.tensor.matmul(
                            t[:, ds(c0, cw)],
                            phatT[0:67, ts(j, 128)],
                            ahatT[:, ds(base + c0, cw)],
                            start=True,
                            stop=True,
                        )
                # Z = z^2 (- EPS_T on the DVE path)
                zsq = zsqpool.tile([128, CS], F32, name="zsq")
                # spread DVE squares so no single block doubles up DVE work
                dve_half = {0: None, 1: 1, 2: None, 3: 0}[j % 4]
                for hi, (t, base) in enumerate(((tpsA, 0), (tpsB, HALF))):
                    if hi == dve_half:
                        nc.vector._custom_dve(
                            sq_bias, out=zsq[:, ds(base, HALF)], in0=t[:],
                            s0=0.0,
                        )
                    else:
                        nc.scalar.activation(
                            zsq[:, ds(base, HALF)], t[:], AF.Square
                        )
                # sbar = sqrt(z^2 - EPS_T)
                sw = swpool.tile([128, CS], F32, name="sw")
                nc.scalar.activation(sw[:], zsq[:], AF.Sqrt, bias=biasme[:])
                # u = max(W*(z - sbar), 1) = max(e^{g-d}, 1)
                u = upool.tile([128, CS], BF16, name="u")
                for t, base in ((tpsA, 0), (tpsB, HALF)):
                    nc.vector._custom_dve(
                        clamp_merge, out=u[:, ds(base, HALF)], in0=t[:],
                        in1=sw[:, ds(base, HALF)], s0=wv[:, ds(j, 1)],
                    )
                # pair products: [1250] -> [625] (+15 pad ones) -> 320 -> 160
                p1 = p1_tiles[j % 4]
                peng = nc.gpsimd if j % 2 == 0 else nc.vector
                peng.tensor_mul(p1[:, 0:625], u[:, 0:625], u[:, 625:1250])
                p2 = p2pool.tile([128, 320], BF16, name="p2")
                nc.gpsimd.tensor_mul(p2[:], p1[:, 0:320], p1[:, 320:640])
                p3 = p2pool.tile([128, 160], BF16, name="p3", tag="p3")
                nc.gpsimd.tensor_mul(p3[:], p2[:, 0:160], p2[:, 160:320])
                nc.gpsimd.tensor_mul(
                    p3buf[:, j, :], p3[:, 0:80], p3[:, 80:160]
                )

            # ---------------- phase B: ln + row-sum accumulate ----------
            # single fused ln spanning ALL blocks -> runs once after the
            # last block; accum_out gives the per-partition hinge sum
            lt = zsqpool.tile([128, NBLK, 80], BF16, name="lnout", tag="lnout")
            nc.scalar.activation(
                lt[:], p3buf[:], AF.Ln, accum_out=hacc[:, ds(0, 1)]
            )

            # ---------------- final: per-partition sums to host ----------
            nc.sync.dma_start(out_d[:], hacc[:, ds(0, 1)])


def _get_nc():
    if "nc" not in _CACHE:
        _CACHE["nc"] = _build()
    return _CACHE["nc"]


def kernel(pred_embs, target_embs, all_embs):
    pred = np.ascontiguousarray(np.asarray(pred_embs, dtype=np.float32))
    targ = np.ascontiguousarray(np.asarray(target_embs, dtype=np.float32))
    alls = np.ascontiguousarray(np.asarray(all_embs, dtype=np.float32))

    nc = _get_nc()
    in_maps = [
        {"pred": pred, "targ": targ, "alls": alls[c * CS:(c + 1) * CS]}
        for c in range(NCORES)
    ]
    res = run_bass_kernel_spmd(nc, in_maps, list(range(NCORES)))
    hinge = sum(float(r["partial"].sum()) for r in res.results)
    loss = (hinge - MARGIN * B) / B
    return np.float32(loss)


if __name__ == "__main__":
    rng = np.random.RandomState(0)

    def ball(rng, n):
        v = rng.randn(n, D).astype(np.float32)
        v /= np.linalg.norm(v, axis=1, keepdims=True) + 1e-8
        r = rng.rand(n, 1).astype(np.float32) * 0.9
        return v * r

    p = ball(rng, B)
    t = ball(rng, B)
    a = ball(rng, C)
    print(kernel(pred_embs=p, target_embs=t, all_embs=a))


# revision 37
# speedup vs baseline: 1.1397x; 1.0241x over previous
import sys

sys.path.insert(0, "/opt/trn_rl_repo")

import math

import numpy as np

import concourse.bacc as bacc
import concourse.bass as bass
import concourse.mybir as mybir
import concourse.tile as tile
from concourse.bass import ds, ts
from concourse.bass_utils import run_bass_kernel_spmd
from concourse.masks import make_identity

B, C, D = 4096, 10000, 64
NCORES = 8
CS = C // NCORES            # 1250 classes per core
NBLK = B // 128             # 32 row blocks of 128
PA = 125                    # partition rows per all_embs block
JA = CS // PA               # 10
MARGIN = 0.1
EM = math.exp(MARGIN)
EPS_T = 1.0 - 1e-5          # sqrt arg = tau0^2 - EPS_T*W^2; z~1 noise gives
                            # sqrt(<0)=NaN which the DVE max drops to u=1
NGRP = 8                    # phase-B ln instructions (each spans all blocks)
LNW = 160 // NGRP           # inner cols per ln instruction
P1W = 640                   # 512 + 113 merged pairs + 15 pad cols of 1.0
SUBP = ((0, 512), (512, 512), (1024, 226))   # psum sub-panels (1 bank each)

F32 = mybir.dt.float32
F32R = mybir.dt.float32r
BF16 = mybir.dt.bfloat16
AF = mybir.ActivationFunctionType
ALU = mybir.AluOpType
PSUM = bass.MemorySpace.PSUM

_CACHE = {}

# ---------------------------------------------------------------------------
# custom DVE ops
# ---------------------------------------------------------------------------


def _register_custom_ops():
    import concourse.dve_ops as dve_ops
    from concourse.dve_ops import OPS, CUSTOM_DVE_SPECS, DveOp
    from concourse.dve_spec import Spec, Src0, Src1, C0, One, maxx, sq, lower
    from concourse.dve_uop import DveOpSpec
    from concourse.dve_table_gen import dve_ver_for

    if "SUBCLAMP_W_PEH" in CUSTOM_DVE_SPECS:
        return (
            dve_ops._PEH_SQUARE_BIAS,  # type: ignore[attr-defined]
            dve_ops._PEH_CLAMP_MERGE,  # type: ignore[attr-defined]
        )

    def mk(name, spec, rd1):
        row = dve_ops._CUSTOM_DVE_ROW_BASE + len(OPS)
        shas = {}
        for ver in ("v3", "v4"):
            try:
                tmp = DveOpSpec(
                    name=name, opcode=row, uops=lower(spec, ver=ver), rd1_en=rd1
                )
                shas[ver] = tmp.sha(ver)
            except Exception:
                pass
        op = DveOp(name, spec, subdim=False, uops_sha=shas)
        OPS.append(op)
        CUSTOM_DVE_SPECS[name] = spec
        dve_ops._SUB_OPCODE_FOR_NAME[name] = row
        return op

    sq_bias = mk(
        "SQUARE_BIAS_PEH",
        Spec(
            body=sq(Src0) - C0,
            reference=lambda in0, in1, s0, s1, imm2: (
                in0.astype(np.float32) * in0.astype(np.float32) - s0
            ),
        ),
        rd1=False,
    )

    def _sc_ref(in0, in1, s0, s1, imm2):
        v = (in0.astype(np.float32) - in1.astype(np.float32)) * s0
        return np.maximum(np.nan_to_num(v, nan=0.0), 1.0)

    clamp_merge = mk(
        "SUBCLAMP_W_PEH",
        Spec(body=maxx((Src0 - Src1) * C0, One), reference=_sc_ref),
        rd1=True,
    )
    dve_ops._PEH_SQUARE_BIAS = sq_bias  # type: ignore[attr-defined]
    dve_ops._PEH_CLAMP_MERGE = clamp_merge  # type: ignore[attr-defined]
    return sq_bias, clamp_merge


def _patch_act_tables():
    """Pin Square/Sqrt to sqrt_and_others and Ln to natural_log by removing
    them from every other set. Only membership changes; list order (and so
    act_func_set_id) is untouched, and the pinned sets genuinely contain the
    functions on hardware."""
    if getattr(bacc, "_peh_act_patch", False):
        return
    orig = bacc.get_activation_tables

    def patched(arch):
        tabs = {k: set(v) for k, v in orig(arch).items()}
        for name, funcs in tabs.items():
            if name != "sqrt_and_others":
                funcs.discard(AF.Square)
                funcs.discard(AF.Sqrt)
            if name != "natural_log":
                funcs.discard(AF.Ln)
        return tabs

    bacc.get_activation_tables = patched
    bacc._peh_act_patch = True


# ---------------------------------------------------------------------------
# kernel body
# ---------------------------------------------------------------------------


def _build():
    sq_bias, clamp_merge = _register_custom_ops()
    _patch_act_tables()
    nc = bacc.Bacc(None, target_bir_lowering=False)
    pred_d = nc.declare_dram_parameter("pred", [B, D], F32, isOutput=False)
    targ_d = nc.declare_dram_parameter("targ", [B, D], F32, isOutput=False)
    alls_d = nc.declare_dram_parameter("alls", [CS, D], F32, isOutput=False)
    out_d = nc.declare_dram_parameter("partial", [128, 1], F32, isOutput=True)

    with tile.TileContext(nc) as tc:
        _body(nc, tc, pred_d, targ_d, alls_d, out_d, sq_bias, clamp_merge)
    nc.compile()
    return nc


def _body(nc, tc, pred_d, targ_d, alls_d, out_d, sq_bias, clamp_merge):
    with (
        tc.tile_pool(name="persist", bufs=1) as persist,
        tc.tile_pool(name="prep", bufs=1) as prep,
    ):
        # The 67-row augmented matmul gives z = cosh(d) per (row, class).
        # sbar = sqrt(z^2 - EPS_T) ~ sinh(d), and one fused DVE op computes
        # u = max(W*(z - sbar), 1) = max(e^{g-d}, 1), so each hinge term is
        # ln(u); products of u's shrink the final Ln pass 8x.
        phatT = persist.tile([128, B], BF16)
        ahatT = persist.tile([67, CS], BF16)
        hacc = persist.tile([128, NGRP], F32)
        ident = persist.tile([128, 128], BF16)
        p3buf = persist.tile([128, NBLK, 80], BF16)

        make_identity(nc, ident[:])
        biasm1 = persist.tile([128, 1], F32)
        nc.vector.memset(biasm1[:], -(1.0 - 2e-7))
        biasme = persist.tile([128, 1], F32)
        nc.vector.memset(biasme[:], -EPS_T)

        # ---------------- loads ----------------
        prednat = prep.tile([128, NBLK, D], F32)
        targnat = prep.tile([128, NBLK, D], F32)
        allnat = prep.tile([PA, JA, D], F32)
        nc.sync.dma_start(allnat[:], alls_d[:].rearrange("(j p) d -> p j d", p=PA))
        PCHUNKS = ((0, 12), (12, 20))
        for c0, cn in PCHUNKS:
            nc.sync.dma_start(
                prednat[:, ds(c0, cn), :],
                pred_d[ds(c0 * 128, cn * 128), :].rearrange(
                    "(j p) d -> p j d", p=128
                ),
            )
            nc.scalar.dma_start(
                targnat[:, ds(c0, cn), :],
                targ_d[ds(c0 * 128, cn * 128), :].rearrange(
                    "(j p) d -> p j d", p=128
                ),
            )

        with tc.tile_pool(name="prep_ps", bufs=2, space=PSUM) as prep_ps:
            # ---------------- ahat path (gates all matmuls) ----------------
            asq = prep.tile([PA, JA, D], F32)
            an = prep.tile([PA, JA], F32)
            beta = prep.tile([PA, JA], F32)
            tmpa = prep.tile([PA, JA], F32)
            nc.vector.tensor_mul(asq[:], allnat[:], allnat[:])
            nc.vector.tensor_reduce(an[:], asq[:], mybir.AxisListType.X, ALU.add)
            nc.vector.tensor_scalar(tmpa[:], an[:], -1.0, 1.0, ALU.mult, ALU.add)
            nc.vector.reciprocal(beta[:], tmpa[:])

            ahatnat = prep.tile([PA, JA, 67], BF16)
            for j in range(JA):
                nc.vector.tensor_scalar_mul(
                    ahatnat[:, j, 0:64], allnat[:, j, :], beta[:, ds(j, 1)]
                )
            nc.vector.tensor_copy(ahatnat[:, :, 64], beta[:])
            nc.vector.tensor_mul(tmpa[:], beta[:], an[:])
            nc.vector.tensor_copy(ahatnat[:, :, 65], tmpa[:])
            nc.vector.memset(ahatnat[:, :, 66], 1.0)
            for jj in range(JA):
                pt = prep_ps.tile([67, 128], BF16)
                nc.tensor.transpose(
                    pt[0:67, 0:PA], ahatnat[:, jj, :], ident[0:PA, 0:PA]
                )
                nc.vector.tensor_copy(ahatT[:, ts(jj, PA)], pt[0:67, 0:PA])

            # ---------------- pn/tn, alpha, W = e^{dc+m} ----------------
            sq = prep.tile([128, NBLK, D], F32)
            pn = prep.tile([128, NBLK], F32)
            tn = prep.tile([128, NBLK], F32)
            alpha = prep.tile([128, NBLK], F32)
            alphat = prep.tile([128, NBLK], F32)
            tmp = prep.tile([128, NBLK], F32)



            sqt = prep.tile([128, NBLK, D], F32, name="sqt")
            s2c = prep.tile([128, NBLK], F32)
            zc = prep.tile([128, NBLK], F32)
            zzc = prep.tile([128, NBLK], F32)
            rc = prep.tile([128, NBLK], F32)
            wv = prep.tile([128, NBLK], F32)
            phatnat = prep.tile([128, NBLK, 128], BF16)
            nc.gpsimd.memset(phatnat[:], 0.0)
            for c0, cn in PCHUNKS:
                hs = ds(c0, cn)
                nc.vector.tensor_mul(
                    sq[:, hs, :], prednat[:, hs, :], prednat[:, hs, :]
                )
                nc.vector.tensor_reduce(
                    pn[:, hs], sq[:, hs, :], mybir.AxisListType.X, ALU.add
                )
                nc.gpsimd.tensor_mul(
                    sqt[:, hs, :], targnat[:, hs, :], targnat[:, hs, :]
                )
                nc.vector.tensor_reduce(
                    tn[:, hs], sqt[:, hs, :], mybir.AxisListType.X, ALU.add
                )
                nc.vector.tensor_scalar(
                    tmp[:, hs], pn[:, hs], -1.0, 1.0, ALU.mult, ALU.add
                )
                nc.vector.reciprocal(alpha[:, hs], tmp[:, hs])
                nc.vector.tensor_scalar(
                    tmp[:, hs], tn[:, hs], -1.0, 1.0, ALU.mult, ALU.add
                )
                nc.vector.reciprocal(alphat[:, hs], tmp[:, hs])
                nc.vector.tensor_sub(
                    sq[:, hs, :], prednat[:, hs, :], targnat[:, hs, :]
                )
                nc.vector.tensor_mul(sq[:, hs, :], sq[:, hs, :], sq[:, hs, :])
                nc.vector.tensor_reduce(
                    s2c[:, hs], sq[:, hs, :], mybir.AxisListType.X, ALU.add
                )
                nc.vector.tensor_mul(s2c[:, hs], s2c[:, hs], alpha[:, hs])
                nc.vector.tensor_mul(s2c[:, hs], s2c[:, hs], alphat[:, hs])
                nc.vector.tensor_scalar(
                    zc[:, hs], s2c[:, hs], 2.0, 1.0, ALU.mult, ALU.add
                )
                nc.vector.tensor_mul(zzc[:, hs], zc[:, hs], zc[:, hs])
                nc.scalar.activation(
                    rc[:, hs], zzc[:, hs], AF.Sqrt, bias=biasm1[:]
                )
                nc.vector.tensor_add(wv[:, hs], zc[:, hs], rc[:, hs])
                nc.vector.tensor_scalar_mul(wv[:, hs], wv[:, hs], EM)
                # phat features + transposes for this half's blocks
                nc.vector.tensor_mul(tmp[:, hs], alpha[:, hs], pn[:, hs])
                nc.vector.tensor_scalar_mul(phatnat[:, hs, 64], tmp[:, hs], 2.0)
                nc.vector.tensor_scalar_mul(
                    phatnat[:, hs, 65], alpha[:, hs], 2.0
                )
                nc.vector.memset(phatnat[:, hs, 66], 1.0)
                for j in range(c0, c0 + cn):
                    nc.gpsimd.tensor_scalar(
                        phatnat[:, j, 0:64], prednat[:, j, :],
                        alpha[:, ds(j, 1)], -4.0, ALU.mult, ALU.mult,
                    )
                    nc.sync.dma_start_transpose(
                        phatT[:, ts(j, 128)], phatnat[:, j, :]
                    )

        # ---------------- phase A ----------------
        with (
            tc.tile_pool(name="mma", bufs=3, space=PSUM) as pa,
            tc.tile_pool(name="mmc", bufs=2, space=PSUM) as pc,
            tc.tile_pool(name="zsq", bufs=6) as zsqpool,
            tc.tile_pool(name="sw", bufs=6) as swpool,
            tc.tile_pool(name="ubuf", bufs=6) as upool,
            tc.tile_pool(name="p1", bufs=3) as p1pool,
            tc.tile_pool(name="p2", bufs=4) as p2pool,
        ):
            p1_tiles = []
            for i in range(4):
                t = p1pool.tile([128, P1W], BF16, name=f"p1_{i}", tag=f"p1_{i}")
                nc.vector.memset(t[:, 625:P1W], 1.0)
                p1_tiles.append(t)

            HALF = 625
            for j in range(NBLK):
                tpsAB = pa.tile([128, 1024], F32, name="tpsAB", tag="tpsAB")
                tpsC = pc.tile([128, 226], F32, name="tpsC", tag="tpsC")
                for c0 in (0, 512):
                    nc.tensor.matmul(
                        tpsAB[:, ds(c0, 512)],
                        phatT[0:67, ts(j, 128)],
                        ahatT[:, ds(c0, 512)],
                        start=True,
                        stop=True,
                    )
                nc.tensor.matmul(
                    tpsC[:],
                    phatT[0:67, ts(j, 128)],
                    ahatT[:, ds(1024, 226)],
                    start=True,
                    stop=True,
                )
                # Z = z^2; spread the DVE square across blocks
                zsq = zsqpool.tile([128, CS], F32, name="zsq")
                panels = ((tpsAB, 0, 1024), (tpsC, 1024, 226))
                dve_panel = {0: None, 1: 0, 2: None, 3: 1}[j % 4]
                for pi, (t, base, cw) in enumerate(panels):
                    if pi == dve_panel:
                        nc.vector._custom_dve(
                            sq_bias, out=zsq[:, ds(base, cw)], in0=t[:],
                            s0=0.0,
                        )
                    else:
                        nc.scalar.activation(
                            zsq[:, ds(base, cw)], t[:], AF.Square
                        )
                # sbar = sqrt(z^2 - EPS_T)
                sw = swpool.tile([128, CS], F32, name="sw")
                nc.scalar.activation(sw[:], zsq[:], AF.Sqrt, bias=biasme[:])
                # u = max(W*(z - sbar), 1) = max(e^{g-d}, 1)
                u = upool.tile([128, CS], BF16, name="u")
                for t, base, cw in panels:
                    nc.vector._custom_dve(
                        clamp_merge, out=u[:, ds(base, cw)], in0=t[:],
                        in1=sw[:, ds(base, cw)], s0=wv[:, ds(j, 1)],
                    )
                # pair products: [1250] -> [625] (+15 pad ones) -> 320 -> 160
                p1 = p1_tiles[j % 4]
                peng = nc.gpsimd if j % 2 == 0 else nc.vector
                peng.tensor_mul(p1[:, 0:625], u[:, 0:625], u[:, 625:1250])
                p2 = p2pool.tile([128, 320], BF16, name="p2")
                nc.gpsimd.tensor_mul(p2[:], p1[:, 0:320], p1[:, 320:640])
                p3 = p2pool.tile([128, 160], BF16, name="p3", tag="p3")
                nc.gpsimd.tensor_mul(p3[:], p2[:, 0:160], p2[:, 160:320])
                nc.gpsimd.tensor_mul(
                    p3buf[:, j, :], p3[:, 0:80], p3[:, 80:160]
                )

            # ---------------- phase B: ln + row-sum accumulate ----------
            # single fused ln spanning ALL blocks -> runs once after the
            # last block; accum_out gives the per-partition hinge sum
            lt = zsqpool.tile([128, NBLK, 80], BF16, name="lnout", tag="lnout")
            nc.scalar.activation(
                lt[:], p3buf[:], AF.Ln, accum_out=hacc[:, ds(0, 1)]
            )

            # ---------------- final: per-partition sums to host ----------
            nc.sync.dma_start(out_d[:], hacc[:, ds(0, 1)])


def _get_nc():
    if "nc" not in _CACHE:
        _CACHE["nc"] = _build()
    return _CACHE["nc"]


def kernel(pred_embs, target_embs, all_embs):
    pred = np.ascontiguousarray(np.asarray(pred_embs, dtype=np.float32))
    targ = np.ascontiguousarray(np.asarray(target_embs, dtype=np.float32))
    alls = np.ascontiguousarray(np.asarray(all_embs, dtype=np.float32))

    nc = _get_nc()
    in_maps = [
        {"pred": pred, "targ": targ, "alls": alls[c * CS:(c + 1) * CS]}
        for c in range(NCORES)
    ]
    res = run_bass_kernel_spmd(nc, in_maps, list(range(NCORES)))
    hinge = sum(float(r["partial"].sum()) for r in res.results)
    loss = (hinge - MARGIN * B) / B
    return np.float32(loss)


if __name__ == "__main__":
    rng = np.random.RandomState(0)

    def ball(rng, n):
        v = rng.randn(n, D).astype(np.float32)
        v /= np.linalg.norm(v, axis=1, keepdims=True) + 1e-8
        r = rng.rand(n, 1).astype(np.float32) * 0.9
        return v * r

    p = ball(rng, B)
    t = ball(rng, B)
    a = ball(rng, C)
    print(kernel(pred_embs=p, target_embs=t, all_embs=a))
